# revision 12
# baseline (speedup 1.0000x reference)
"""Trainium2 Bass kernel for FFT-conv1d (= valid cross-correlation conv1d).

Reference computes, for x[N=64, C=64, W=4096], w[F=64, C=64, WW=16], b[F=64]:
    out[n, f, t] = sum_{c, j} x[n, c, t + j] * w[f, c, j] + b[f],  t in [0, 4081)

Strategy (v5, 2-parallel fast-FIR / Karatsuba + phase-block layout):
  - Data-parallel: shard N across 8 NeuronCores (8 samples per core).
  - Split outputs and taps by parity:  u[m]=x[2m], v[m]=x[2m+1],
    g[i]=h[2i], k[i]=h[2i+1] (length-8 subfilters).  With
      A = g*u,  B = k*v,  C = (g+k)*(v + u(+1))
    the outputs are   y[2m] = A[m] + B[m],  y[2m+1] = C[m] - A[m+1] - B[m].
    3 length-8 correlations instead of 4  ->  25% fewer TensorE cycles.
  - Each correlation runs as 4 accumulating K=128 matmuls: SBUF rows 0-63
    hold phase[c, m], rows 64-127 hold phase[c, m+1] (block layout; all
    rhs access patterns are contiguous).  Two samples occupy PE column
    groups 0/64 and stream concurrently.
  - u/v/s phases are precomputed on host (s = v + u(+1)); the row 64-127
    shifted halves are built by SBUF->SBUF shift DMAs on the gpsimd
    SWDGE queue, except pair 0's xu, which loads straight from HBM with
    a +1 element offset so the first matmul never waits on a 2-hop
    load->shift chain.  Pair-0 loads are split in two chunks issued
    critical-first; HAM warm-up matmuls keep the PE clock at 8/8.
  - Combines: ACT evacuates B with fused bias (Bs = B + b); DVE does
    y_even = A + Bs, t = C - Bs, y_odd = (t + 2b) - A' -- 3 DVE ops +
    1 ACT op per 1022 outputs, written bf16 (host upcasts to fp32).
"""

import numpy as np

N, C, W = 64, 64, 4096
F, WW = 64, 16
OUT_W = W - WW + 1  # 4081
N_CORES = 8
NPC = N // N_CORES  # samples per core = 8
PW = 2056           # padded phase width (max col read = 1533+6+511 = 2050)
M0S = (0, 511, 1022, 1533)   # m-tile bases (511-stride, 512-wide banks)
CH = (0, 1040, 2056)         # pair-0 load/shift chunk boundaries

_CACHE = {}


def _build_nc():
    from contextlib import ExitStack

    import concourse.bacc as bacc
    import concourse.mybir as mybir
    import concourse.tile as tile

    f32 = mybir.dt.float32
    bf16 = mybir.dt.bfloat16
    ADD = mybir.AluOpType.add
    SUB = mybir.AluOpType.subtract

    nc = bacc.Bacc(
        "TRN2", target_bir_lowering=False, debug=False, num_devices=N_CORES
    )
    xu_d = nc.dram_tensor("xu", [NPC, C, PW], bf16, kind="ExternalInput").ap()
    xv_d = nc.dram_tensor("xv", [NPC, C, PW], bf16, kind="ExternalInput").ap()
    ss_d = nc.dram_tensor("ss", [NPC, C, PW], bf16, kind="ExternalInput").ap()
    w_d = nc.dram_tensor("wstk", [128, 768], bf16, kind="ExternalInput").ap()
    b1_d = nc.dram_tensor("bias1", [128, 1], f32, kind="ExternalInput").ap()
    b2_d = nc.dram_tensor("bias2", [128, 1], f32, kind="ExternalInput").ap()
    o_d = nc.dram_tensor("out", [NPC, F, OUT_W], bf16, kind="ExternalOutput").ap()

    with tile.TileContext(nc) as tc:
        with ExitStack() as ctx:
            consts = ctx.enter_context(tc.tile_pool(name="consts", bufs=1))
            xpool = ctx.enter_context(tc.tile_pool(name="xs", bufs=4))
            opool = ctx.enter_context(tc.tile_pool(name="osb", bufs=3))
            tpool = ctx.enter_context(tc.tile_pool(name="tmp", bufs=4))
            pspool = ctx.enter_context(
                tc.tile_pool(name="ps", bufs=7, space="PSUM")
            )

            wsb = consts.tile([128, 768], bf16)
            nc.gpsimd.dma_start(out=wsb[:, :], in_=w_d[:, :])
            b1sb = consts.tile([128, 1], f32)
            nc.gpsimd.dma_start(out=b1sb[:, :], in_=b1_d[:, :])
            b2sb = consts.tile([128, 1], f32)
            nc.gpsimd.dma_start(out=b2sb[:, :], in_=b2_d[:, :])

            # HAM warm-up: ~4us of dummy matmuls during the load phase so
            # the PE clock gate is at 8/8 when the first real matmul lands.
            wrm = consts.tile([128, 512], bf16)
            nc.vector.memset(wrm[:, :], 0.0)
            pswm = pspool.tile([128, 512], f32, name="pswm", bufs=1)
            for _ in range(10):
                nc.tensor.matmul(
                    pswm[0:64, :], lhsT=wrm[:, 0:64], rhs=wrm[:, :],
                    start=True, stop=True,
                )

            tiles = {}

            def shift(xt, lo, hi):
                # rows 64-127 <- rows 0-63 shifted one phase-col left
                nc.gpsimd.dma_start(
                    out=xt[64:128, lo:hi], in_=xt[0:64, lo + 1 : hi + 1]
                )

            def emit_loads(p):
                trio = [
                    [
                        xpool.tile([128, PW], bf16, name=f"x{s}{di}")
                        for di in range(3)
                    ]
                    for s in range(2)
                ]
                srcs = (xu_d, xv_d, ss_d)
                if p == 0:
                    # two chunks, critical-first; sample 0 on sync, sample 1
                    # on scalar; xu's shifted half loads from HBM (+1 col)
                    for ci in range(2):
                        lo, hi = CH[ci], CH[ci + 1]
                        for s in range(2):
                            eng = nc.scalar if s == 1 else nc.sync
                            for di, src in enumerate(srcs):
                                xt = trio[s][di]
                                eng.dma_start(
                                    out=xt[0:64, lo:hi], in_=src[2 * p + s, :, lo:hi]
                                )
                                if di == 0:
                                    hi2 = min(hi, PW - 1)
                                    eng.dma_start(
                                        out=xt[64:128, lo:hi2],
                                        in_=src[2 * p + s, :, lo + 1 : hi2 + 1],
                                    )
                    for ci in range(2):
                        # chunk-local shifts: src stays within the same chunk
                        lo = 0 if ci == 0 else CH[1] - 1
                        hi = CH[1] - 1 if ci == 0 else PW - 5
                        for di in (1, 2):
                            for s in range(2):
                                shift(trio[s][di], lo, hi)
                else:
                    for di, src in enumerate(srcs):
                        for s in range(2):
                            nc.sync.dma_start(
                                out=trio[s][di][0:64, :], in_=src[2 * p + s, :, :]
                            )
                    for di in range(3):
                        for s in range(2):
                            shift(trio[s][di], 0, PW - 5)
                tiles[p] = trio

            def combine(osb, psA, psB, psC, m0, lo, hi):
                # columns [lo, hi) of the 512-wide banks -> outputs
                # y[2(m0+j)] and y[2(m0+j)+1] for j in [lo, hi)
                n = hi - lo
                tB = tpool.tile([128, 512], f32)
                nc.scalar.add(tB[:, lo:hi], psB[:, lo:hi], b1sb[:, 0:1])
                nc.vector.tensor_add(
                    osb[:, 2 * (m0 + lo) : 2 * (m0 + lo) + 2 * n : 2],
                    psA[:, lo:hi],
                    tB[:, lo:hi],
                )
                tT = tpool.tile([128, 512], f32)
                nc.vector.tensor_sub(tT[:, 0:n], psC[:, lo:hi], tB[:, lo:hi])
                nc.vector.scalar_tensor_tensor(
                    osb[:, 2 * (m0 + lo) + 1 : 2 * (m0 + lo) + 2 * n + 1 : 2],
                    tT[:, 0:n],
                    b2sb[:, 0:1],
                    psA[:, lo + 1 : hi + 1],
                    ADD,
                    SUB,
                )

            def emit_compute(p):
                trio = tiles.pop(p)
                osb = opool.tile([128, 4090], bf16)
                for tt, m0 in enumerate(M0S):
                    banks = []
                    for d in range(3):
                        ps = pspool.tile([128, 512], f32, name="ps")
                        for a in range(4):
                            blk = d * 4 + a
                            for s in range(2):
                                nc.tensor.matmul(
                                    ps[64 * s : 64 * (s + 1), :],
                                    lhsT=wsb[:, blk * 64 : (blk + 1) * 64],
                                    rhs=trio[s][d][:, m0 + 2 * a : m0 + 2 * a + 512],
                                    start=(a == 0),
                                    stop=(a == 3),
                                )
                        banks.append(ps)
                    psA, psB, psC = banks
                    last = p == 3 and tt == 3
                    if last:
                        # split the final combine so the tail after the very
                        # last matmul is two short chains instead of one long
                        combine(osb, psA, psB, psC, m0, 0, 256)
                        nc.sync.dma_start(
                            out=o_d[2 * p : 2 * p + 2].flatten_outer_dims()[
                                :, 2 * m0 : 2 * m0 + 512
                            ],
                            in_=osb[:, 2 * m0 : 2 * m0 + 512],
                        )
                        combine(osb, psA, psB, psC, m0, 256, 511)
                        nc.sync.dma_start(
                            out=o_d[2 * p : 2 * p + 2].flatten_outer_dims()[
                                :, 2 * m0 + 512 : OUT_W
                            ],
                            in_=osb[:, 2 * m0 + 512 : OUT_W],
                        )
                    else:
                        combine(osb, psA, psB, psC, m0, 0, 511)
                        lo = 2 * m0
                        hi = min(lo + 1022, OUT_W)
                        eng = nc.sync if (p == 3 and tt == 2) else nc.scalar
                        eng.dma_start(
                            out=o_d[2 * p : 2 * p + 2].flatten_outer_dims()[
                                :, lo:hi
                            ],
                            in_=osb[:, lo:hi],
                        )

            for p in range(4):
                emit_loads(p)
            for p in range(4):
                emit_compute(p)

    nc.compile()
    return nc


def _get_nc():
    if "nc" not in _CACHE:
        _CACHE["nc"] = _build_nc()
    return _CACHE["nc"]


def _host_prep(x, w, b):
    """Phase-split x, build s = v + u(+1), pack subfilter weights."""
    import ml_dtypes

    bf16 = ml_dtypes.bfloat16
    n = x.shape[0]
    u = np.zeros((n, C, PW), dtype=bf16)
    u[:, :, :2048] = x[:, :, 0::2]
    v = np.zeros((n, C, PW), dtype=bf16)
    v[:, :, :2048] = x[:, :, 1::2]
    ss = np.zeros((n, C, PW), dtype=bf16)
    ss[:, :, :2047] = x[:, :, 1:4094:2] + x[:, :, 2:4095:2]

    # wstk[row, (d*4+a)*64 + f]: rows 0-63 tap 4a+{0,1,0&1}, rows 64-127
    # tap 4a+{2,3,2&3} for d = A,B,C
    wA0 = w[:, :, 0::4].transpose(1, 2, 0).reshape(C, 256)   # [c, a*64+f]
    wA1 = w[:, :, 2::4].transpose(1, 2, 0).reshape(C, 256)
    wB0 = w[:, :, 1::4].transpose(1, 2, 0).reshape(C, 256)
    wB1 = w[:, :, 3::4].transpose(1, 2, 0).reshape(C, 256)
    wstk = np.zeros((128, 768), dtype=np.float32)
    wstk[0:64, 0:256] = wA0
    wstk[64:128, 0:256] = wA1
    wstk[0:64, 256:512] = wB0
    wstk[64:128, 256:512] = wB1
    wstk[0:64, 512:768] = wA0 + wB0
    wstk[64:128, 512:768] = wA1 + wB1
    wstk = np.ascontiguousarray(wstk.astype(bf16))
    b1 = np.ascontiguousarray(np.concatenate([b, b]).astype(np.float32).reshape(128, 1))
    b2 = np.ascontiguousarray((2.0 * b1).astype(np.float32))
    return u, v, ss, wstk, b1, b2


def _make_in_maps(x, w, b):
    u, v, ss, wstk, b1, b2 = _host_prep(x, w, b)
    return [
        {
            "xu": np.ascontiguousarray(u[i * NPC : (i + 1) * NPC]),
            "xv": np.ascontiguousarray(v[i * NPC : (i + 1) * NPC]),
            "ss": np.ascontiguousarray(ss[i * NPC : (i + 1) * NPC]),
            "wstk": wstk,
            "bias1": b1,
            "bias2": b2,
        }
        for i in range(N_CORES)
    ]


def kernel(x, w, b):
    from concourse.bass_utils import run_bass_kernel_spmd

    x = np.asarray(x, dtype=np.float32)
    w = np.asarray(w, dtype=np.float32)
    b = np.asarray(b, dtype=np.float32)
    assert x.shape == (N, C, W) and w.shape == (F, C, WW) and b.shape == (F,)

    nc = _get_nc()
    in_maps = _make_in_maps(x, w, b)
    res = run_bass_kernel_spmd(nc, in_maps, core_ids=list(range(N_CORES)))
    out = np.concatenate([np.asarray(r["out"]) for r in res.results], axis=0)
    return out.astype(np.float32)


# revision 14
# speedup vs baseline: 1.0338x; 1.0338x over previous
"""Trainium2 Bass kernel for FFT-conv1d (= valid cross-correlation conv1d).

Reference computes, for x[N=64, C=64, W=4096], w[F=64, C=64, WW=16], b[F=64]:
    out[n, f, t] = sum_{c, j} x[n, c, t + j] * w[f, c, j] + b[f],  t in [0, 4081)

Strategy (v5, 2-parallel fast-FIR / Karatsuba + phase-block layout):
  - Data-parallel: shard N across 8 NeuronCores (8 samples per core).
  - Split outputs and taps by parity:  u[m]=x[2m], v[m]=x[2m+1],
    g[i]=h[2i], k[i]=h[2i+1] (length-8 subfilters).  With
      A = g*u,  B = k*v,  C = (g+k)*(v + u(+1))
    the outputs are   y[2m] = A[m] + B[m],  y[2m+1] = C[m] - A[m+1] - B[m].
    3 length-8 correlations instead of 4  ->  25% fewer TensorE cycles.
  - Each correlation runs as 4 accumulating K=128 matmuls: SBUF rows 0-63
    hold phase[c, m], rows 64-127 hold phase[c, m+1] (block layout; all
    rhs access patterns are contiguous).  Two samples occupy PE column
    groups 0/64 and stream concurrently.
  - u/v/s phases are precomputed on host (s = v + u(+1)); the row 64-127
    shifted halves are built by SBUF->SBUF shift DMAs on the gpsimd
    SWDGE queue, except pair 0's xu, which loads straight from HBM with
    a +1 element offset so the first matmul never waits on a 2-hop
    load->shift chain.  Pair-0 loads are split in two chunks issued
    critical-first; HAM warm-up matmuls keep the PE clock at 8/8.
  - Combines: ACT evacuates B with fused bias (Bs = B + b); DVE does
    y_even = A + Bs, t = C - Bs, y_odd = (t + 2b) - A' -- 3 DVE ops +
    1 ACT op per 1022 outputs, written bf16 (host upcasts to fp32).
"""

import numpy as np

N, C, W = 64, 64, 4096
F, WW = 64, 16
OUT_W = W - WW + 1  # 4081
N_CORES = 8
NPC = N // N_CORES  # samples per core = 8
PW = 2056           # padded phase width (max col read = 1533+6+511 = 2050)
M0S = (0, 511, 1022, 1533)   # m-tile bases (511-stride, 512-wide banks)
CH = (0, 1040, 2056)         # pair-0 load/shift chunk boundaries

_CACHE = {}


def _build_nc():
    from contextlib import ExitStack

    import concourse.bacc as bacc
    import concourse.mybir as mybir
    import concourse.tile as tile

    f32 = mybir.dt.float32
    bf16 = mybir.dt.bfloat16
    ADD = mybir.AluOpType.add
    SUB = mybir.AluOpType.subtract

    nc = bacc.Bacc(
        "TRN2", target_bir_lowering=False, debug=False, num_devices=N_CORES
    )
    xu_d = nc.dram_tensor("xu", [NPC, C, PW], bf16, kind="ExternalInput").ap()
    xv_d = nc.dram_tensor("xv", [NPC, C, PW], bf16, kind="ExternalInput").ap()
    ss_d = nc.dram_tensor("ss", [NPC, C, PW], bf16, kind="ExternalInput").ap()
    w_d = nc.dram_tensor("wstk", [128, 768], bf16, kind="ExternalInput").ap()
    b1_d = nc.dram_tensor("bias1", [128, 1], f32, kind="ExternalInput").ap()
    b2_d = nc.dram_tensor("bias2", [128, 1], f32, kind="ExternalInput").ap()
    o_d = nc.dram_tensor("out", [NPC, F, OUT_W], bf16, kind="ExternalOutput").ap()

    with tile.TileContext(nc) as tc:
        with ExitStack() as ctx:
            consts = ctx.enter_context(tc.tile_pool(name="consts", bufs=1))
            xpool = ctx.enter_context(tc.tile_pool(name="xs", bufs=4))
            opool = ctx.enter_context(tc.tile_pool(name="osb", bufs=3))
            tpool = ctx.enter_context(tc.tile_pool(name="tmp", bufs=4))
            pspool = ctx.enter_context(
                tc.tile_pool(name="ps", bufs=8, space="PSUM")
            )

            wsb = consts.tile([128, 768], bf16)
            nc.gpsimd.dma_start(out=wsb[:, :], in_=w_d[:, :])
            b1sb = consts.tile([128, 1], f32)
            nc.gpsimd.dma_start(out=b1sb[:, :], in_=b1_d[:, :])
            b2sb = consts.tile([128, 1], f32)
            nc.gpsimd.dma_start(out=b2sb[:, :], in_=b2_d[:, :])

            # HAM warm-up: ~4us of dummy matmuls during the load phase so
            # the PE clock gate is at 8/8 when the first real matmul lands.
            wrm = consts.tile([128, 512], bf16)
            nc.vector.memset(wrm[:, :], 0.0)
            pswm = pspool.tile([128, 512], f32, name="ps")
            for _ in range(12):
                nc.tensor.matmul(
                    pswm[0:64, :], lhsT=wrm[:, 0:64], rhs=wrm[:, :],
                    start=True, stop=True,
                )

            tiles = {}

            def emit_loads(p):
                trio = [
                    [
                        xpool.tile([128, PW], bf16, name=f"x{s}{di}")
                        for di in range(3)
                    ]
                    for s in range(2)
                ]
                srcs = (xu_d, xv_d, ss_d)
                # both halves of every phase tile load straight from HBM
                # (rows 64-127 from the same rows at +1 col) -- no SBUF->SBUF
                # shift DMAs, whose ~5us completion latency stalled the PE.
                # issue is spread over three queues so no engine spends more
                # than ~20us generating descriptors.
                for s in range(2):
                    if p == 0:
                        eng = nc.scalar if s == 1 else nc.sync
                    else:
                        eng = nc.gpsimd if p == 2 else nc.sync
                    for di, src_ in enumerate(srcs):
                        xt = trio[s][di]
                        eng.dma_start(out=xt[0:64, :], in_=src_[2 * p + s, :, :])
                        eng.dma_start(
                            out=xt[64:128, 0 : PW - 1],
                            in_=src_[2 * p + s, :, 1:PW],
                        )
                tiles[p] = trio

            def combine(osb, psA, psB, psC, m0, lo, hi):
                # columns [lo, hi) of the 512-wide banks -> outputs
                # y[2(m0+j)] and y[2(m0+j)+1] for j in [lo, hi).
                # ACT evacuates B (+bias) and C; gpsimd forms C - B - b in
                # SBUF; DVE only writes the two interleaved output streams.
                n = hi - lo
                tB = tpool.tile([128, 512], f32, name="tB")
                nc.scalar.add(tB[:, lo:hi], psB[:, lo:hi], b1sb[:, 0:1])
                sC = tpool.tile([128, 512], f32, name="sC")
                nc.scalar.copy(sC[:, lo:hi], psC[:, lo:hi])
                nc.vector.tensor_add(
                    osb[:, 2 * (m0 + lo) : 2 * (m0 + lo) + 2 * n : 2],
                    psA[:, lo:hi],
                    tB[:, lo:hi],
                )
                tT = tpool.tile([128, 512], f32, name="tT")
                nc.gpsimd.tensor_sub(tT[:, lo:hi], sC[:, lo:hi], tB[:, lo:hi])
                nc.vector.scalar_tensor_tensor(
                    osb[:, 2 * (m0 + lo) + 1 : 2 * (m0 + lo) + 2 * n + 1 : 2],
                    tT[:, lo:hi],
                    b2sb[:, 0:1],
                    psA[:, lo + 1 : hi + 1],
                    ADD,
                    SUB,
                )

            def emit_compute(p):
                trio = tiles.pop(p)
                osb = opool.tile([128, 4090], bf16)
                for tt, m0 in enumerate(M0S):
                    banks = []
                    for d in range(3):
                        ps = pspool.tile([128, 512], f32, name="ps")
                        for a in range(4):
                            blk = d * 4 + a
                            for s in range(2):
                                nc.tensor.matmul(
                                    ps[64 * s : 64 * (s + 1), :],
                                    lhsT=wsb[:, blk * 64 : (blk + 1) * 64],
                                    rhs=trio[s][d][:, m0 + 2 * a : m0 + 2 * a + 512],
                                    start=(a == 0),
                                    stop=(a == 3),
                                )
                        banks.append(ps)
                    psA, psB, psC = banks
                    last = p == 3 and tt == 3
                    if last:
                        # split the final combine so the tail after the very
                        # last matmul is two short chains instead of one long
                        combine(osb, psA, psB, psC, m0, 0, 256)
                        nc.sync.dma_start(
                            out=o_d[2 * p : 2 * p + 2].flatten_outer_dims()[
                                :, 2 * m0 : 2 * m0 + 512
                            ],
                            in_=osb[:, 2 * m0 : 2 * m0 + 512],
                        )
                        combine(osb, psA, psB, psC, m0, 256, 511)
                        nc.sync.dma_start(
                            out=o_d[2 * p : 2 * p + 2].flatten_outer_dims()[
                                :, 2 * m0 + 512 : OUT_W
                            ],
                            in_=osb[:, 2 * m0 + 512 : OUT_W],
                        )
                    else:
                        combine(osb, psA, psB, psC, m0, 0, 511)
                        lo = 2 * m0
                        hi = min(lo + 1022, OUT_W)
                        eng = nc.sync
                        eng.dma_start(
                            out=o_d[2 * p : 2 * p + 2].flatten_outer_dims()[
                                :, lo:hi
                            ],
                            in_=osb[:, lo:hi],
                        )

            for p in range(4):
                emit_loads(p)
            for p in range(4):
                emit_compute(p)

    nc.compile()
    return nc


def _get_nc():
    if "nc" not in _CACHE:
        _CACHE["nc"] = _build_nc()
    return _CACHE["nc"]


def _host_prep(x, w, b):
    """Phase-split x, build s = v + u(+1), pack subfilter weights."""
    import ml_dtypes

    bf16 = ml_dtypes.bfloat16
    n = x.shape[0]
    u = np.zeros((n, C, PW), dtype=bf16)
    u[:, :, :2048] = x[:, :, 0::2]
    v = np.zeros((n, C, PW), dtype=bf16)
    v[:, :, :2048] = x[:, :, 1::2]
    ss = np.zeros((n, C, PW), dtype=bf16)
    ss[:, :, :2047] = x[:, :, 1:4094:2] + x[:, :, 2:4095:2]

    # wstk[row, (d*4+a)*64 + f]: rows 0-63 tap 4a+{0,1,0&1}, rows 64-127
    # tap 4a+{2,3,2&3} for d = A,B,C
    wA0 = w[:, :, 0::4].transpose(1, 2, 0).reshape(C, 256)   # [c, a*64+f]
    wA1 = w[:, :, 2::4].transpose(1, 2, 0).reshape(C, 256)
    wB0 = w[:, :, 1::4].transpose(1, 2, 0).reshape(C, 256)
    wB1 = w[:, :, 3::4].transpose(1, 2, 0).reshape(C, 256)
    wstk = np.zeros((128, 768), dtype=np.float32)
    wstk[0:64, 0:256] = wA0
    wstk[64:128, 0:256] = wA1
    wstk[0:64, 256:512] = wB0
    wstk[64:128, 256:512] = wB1
    wstk[0:64, 512:768] = wA0 + wB0
    wstk[64:128, 512:768] = wA1 + wB1
    wstk = np.ascontiguousarray(wstk.astype(bf16))
    b1 = np.ascontiguousarray(np.concatenate([b, b]).astype(np.float32).reshape(128, 1))
    b2 = np.ascontiguousarray((2.0 * b1).astype(np.float32))
    return u, v, ss, wstk, b1, b2


def _make_in_maps(x, w, b):
    u, v, ss, wstk, b1, b2 = _host_prep(x, w, b)
    return [
        {
            "xu": np.ascontiguousarray(u[i * NPC : (i + 1) * NPC]),
            "xv": np.ascontiguousarray(v[i * NPC : (i + 1) * NPC]),
            "ss": np.ascontiguousarray(ss[i * NPC : (i + 1) * NPC]),
            "wstk": wstk,
            "bias1": b1,
            "bias2": b2,
        }
        for i in range(N_CORES)
    ]


def kernel(x, w, b):
    from concourse.bass_utils import run_bass_kernel_spmd

    x = np.asarray(x, dtype=np.float32)
    w = np.asarray(w, dtype=np.float32)
    b = np.asarray(b, dtype=np.float32)
    assert x.shape == (N, C, W) and w.shape == (F, C, WW) and b.shape == (F,)

    nc = _get_nc()
    in_maps = _make_in_maps(x, w, b)
    res = run_bass_kernel_spmd(nc, in_maps, core_ids=list(range(N_CORES)))
    out = np.concatenate([np.asarray(r["out"]) for r in res.results], axis=0)
    return out.astype(np.float32)
